# revision 17
# baseline (speedup 1.0000x reference)
"""BasicGNNConv on 8 TRN2 NeuronCores (Bass/Tile).

Math (reference):
    h   = node_feat @ Wn + bn                    # [N, 128]
    e   = edge_feat @ We + be                    # [E, 128]
    m   = h[src] + e
    agg = segment_sum(m, dst) / max(deg, 1)
    out = concat([h, agg]) @ Wc + bc

Linearity rewrite (eliminates all per-edge matmuls; biases folded):
    ht   = node_feat @ Wn                        # no bias
    aggT = rcol * [ (sum ht[src])T + We.T @ (sum ef)T ]      # [feat, slot]
    outT = Wc1.T @ htT_own + Wc2.T @ aggT + bnbeWc2 (x) mcol + bias0 (x) 1

Sharding: edges are assigned to the core that owns their dst node range
(5000 nodes/core) -> per-core segment sums are complete, no collective needed.

v2 changes vs the 395us baseline (trace-driven):
  * Gather batching: the GpSimd DMAGatherAnt descriptor generation was the
    phase-B bottleneck (267us busy, ~90% of the phase-B window) because each
    call carried <=1024 descriptors against a ~1us fixed overhead.  The
    SWDGE carveout is raised to 49152B (3072-desc ring per queue) and each
    (superblock, half) issues ONE gather call (~2.2k descs), lo on queue 0
    and hi on queue 1: 40 calls instead of ~120.
  * fp8 phase B: table rows are [fp8(h) | fp8(h - fp8(h))] (value+residual,
    256B -- the same gather size as fp16, since 256B is the dma_gather
    minimum), and the edge-feature stream is fp8 (halves that stream).  The
    per-tile segment-sum matmuls run as fp8 DoubleRow (2 edge-tiles per
    pass): 3 matmuls (h-val, h-res, ef) per PAIR of tiles.  The residual
    term restores the h path to ~fp16 accuracy; measured end-to-end rel err
    ~1e-2 vs the 2e-2 gate.
  * Merged lo/hi passes: one sweep over 20 superblocks; each superblock
    gathers its lo-src edges (from the lo table) and hi-src edges (hi
    table) back to back, so the lo-partial flush/reload of the two-pass
    scheme disappears.
  * outT is fp16 (host upcasts); rcolR is fp16.

(Collectives were probed for a sharded phase A -- an intra-chip AllGather
completes ~90us after trigger in this runtime, erasing the benefit, so the
table build stays replicated per-core.)
"""
import numpy as np
import ml_dtypes

import concourse.bacc as bacc
import concourse.mybir as mybir
import concourse.tile as tile
from concourse.tile_rust import add_dep_helper
from concourse.bass_utils import run_bass_kernel_spmd

N = 40000
E = 640000
D = 128          # OUT_DIM == EDGE_DIM
ND = 256         # NODE_DIM
C = 8            # cores
NPC = N // C     # 5000 nodes per core
BLK = 125        # nodes per dst block
NB = NPC // BLK  # 40 blocks per core
SBLK = 2         # blocks per superblock
NSB = NB // SBLK
NLO = 16384      # nodes in the lo gather table (8 phase-A chunks)
TLO = NLO // 128          # 128 t-columns
NHICAP = 23680            # 185 * 128 (capacity; real nodes 23616)
THI = NHICAP // 128       # 185
CH = 2048        # phase A chunk (nodes)
NCH = (N + CH - 1) // CH  # 20 (last chunk 1088 nodes)
CH2 = 500        # phase A2 chunk (own nodes)
PAD_COL = 127    # trash column in the 128-wide S window (>= BLK)
NQ = 4           # SWDGE queues
SCRATCH = 16384  # dynamic DMA scratch (HW ring is 1024 descs regardless)
SGRP = 8         # S tiles per IS_EQ call
GT = 2           # tiles per gather call (256 descs -> 3 in flight per ring,
                 # hiding the ~4.6us ring-reclaim round trip)

LAST_EXEC_NS = None
LAST_RESULTS = None

f16 = np.float16
f8 = ml_dtypes.float8_e4m3    # TRN fp8e4 flavor


def _wrap_idx16(arr):
    """[L] -> [128, L//16] int16 wrapped layout (pos i at [i%16, i//16]),
    replicated across the 8 GPSIMD core partition groups."""
    w = arr.astype(np.int16).reshape(-1, 16).T
    return np.ascontiguousarray(np.tile(w, (8, 1)))


def _build_graph(T_list):
    """T_list[b] = (T_lo, T_hi) tiles per block; stream order is
    (superblock, half, block-within-superblock)."""
    nc = bacc.Bacc(
        None, target_bir_lowering=False, debug=False,
        num_swdge_queues=NQ, dynamic_dma_scratch_size=SCRATCH,
    )
    f32, i16, fh = mybir.dt.float32, mybir.dt.int16, mybir.dt.float16
    fp8 = mybir.dt.float8e4
    DR = mybir.MatmulPerfMode.DoubleRow

    T_tot = sum(tl + th for tl, th in T_list)
    L = T_tot * 128

    nfT_p = nc.declare_dram_parameter("nfT", [ND, N], fh, isOutput=False)
    nfTo_p = nc.declare_dram_parameter("nfTo", [ND, NPC], fh, isOutput=False)
    Wn_p = nc.declare_dram_parameter("Wn16", [ND, D], fh, isOutput=False)
    We_p = nc.declare_dram_parameter("We16", [D, D], fh, isOutput=False)
    Wc1_p = nc.declare_dram_parameter("Wc116", [D, D], fh, isOutput=False)
    Wc2_p = nc.declare_dram_parameter("Wc216", [D, D], fh, isOutput=False)
    L2_p = nc.declare_dram_parameter("L2", [2, D], fh, isOutput=False)
    R2_p = nc.declare_dram_parameter("R2", [2, NB * 128], fh, isOutput=False)
    gidx_p = nc.declare_dram_parameter("gidx", [128, L // 16], i16, isOutput=False)
    dstf_p = nc.declare_dram_parameter("dstf", [128, T_tot], fh, isOutput=False)
    rcolR_p = nc.declare_dram_parameter("rcolR", [128, NB * 128], fh, isOutput=False)
    ef_p = nc.declare_dram_parameter("ef", [128, T_tot, D], fp8, isOutput=False)
    outT_p = nc.declare_dram_parameter("outT", [D, NPC], fh, isOutput=True)

    # table rows are 256B: [fp8 h | fp8 residual]
    htab = [
        nc.dram_tensor("htab_lo", [NLO, 2 * D], fp8),
        nc.dram_tensor("htab_hi", [NHICAP, 2 * D], fp8),
    ]

    # span bookkeeping: stream is (sb, half, block-in-sb)
    sb_T = [[0] * NSB, [0] * NSB]
    for h in range(2):
        for sb in range(NSB):
            sb_T[h][sb] = sum(T_list[sb * SBLK + j][h] for j in range(SBLK))
    TSBMAX = max(sb_T[0][sb] + sb_T[1][sb] for sb in range(NSB))
    # offset of (sb, half) span in the global tile stream
    span_off = {}
    off = 0
    for sb in range(NSB):
        for h in range(2):
            span_off[(sb, h)] = off
            off += sb_T[h][sb]
    assert off == T_tot

    with tile.TileContext(nc) as tc:
        with (
            tc.tile_pool(name="const", bufs=1) as cpool,
            tc.tile_pool(name="tabs", bufs=1) as tpool,
        ):
            # ---- constants / weights in SBUF ----
            iota_i = cpool.tile([128, 128], mybir.dt.int32)
            nc.gpsimd.iota(iota_i[:], pattern=[[1, 128]], base=0, channel_multiplier=0)
            iota8 = cpool.tile([128, SGRP, 128], fh)
            for jj in range(SGRP):
                nc.vector.tensor_copy(iota8[:, jj, :], iota_i[:])

            Wn_sb = cpool.tile([128, ND // 128, D], fh)
            nc.sync.dma_start(out=Wn_sb[:], in_=Wn_p[:].rearrange("(k p) d -> p k d", p=128))
            gidx_sb = cpool.tile([128, L // 16], i16)
            nc.sync.dma_start(out=gidx_sb[:], in_=gidx_p[:])
            dstf_sb = cpool.tile([128, T_tot], fh)
            nc.sync.dma_start(out=dstf_sb[:], in_=dstf_p[:])
            We_sb = cpool.tile([128, D], fh)
            nc.scalar.dma_start(out=We_sb[:], in_=We_p[:])
            Wc1_sb = cpool.tile([128, D], fh)
            nc.scalar.dma_start(out=Wc1_sb[:], in_=Wc1_p[:])
            Wc2_sb = cpool.tile([128, D], fh)
            nc.scalar.dma_start(out=Wc2_sb[:], in_=Wc2_p[:])
            L2_sb = cpool.tile([2, D], fh)
            nc.scalar.dma_start(out=L2_sb[:], in_=L2_p[:])
            R2_sb = cpool.tile([2, NB, 128], fh)
            nc.scalar.dma_start(out=R2_sb[:], in_=R2_p[:].rearrange("p (b j) -> p b j", j=128))
            rcolR_sb = cpool.tile([128, NB, 128], fh)
            nc.scalar.dma_start(
                out=rcolR_sb[:], in_=rcolR_p[:].rearrange("p (b j) -> p b j", j=128)
            )

            hownT = tpool.tile([128, NB, BLK], fh)   # ht.T of own nodes

            with (
                tc.tile_pool(name="phA", bufs=3) as apool,
                tc.tile_pool(name="psA", bufs=2, space="PSUM") as apsum,
                tc.tile_pool(name="phB", bufs=3) as bpool,
                tc.tile_pool(name="phS", bufs=3) as spool,
                tc.tile_pool(name="phC", bufs=2) as cpl,
                tc.tile_pool(name="phO", bufs=2) as opool,
                tc.tile_pool(name="psB", bufs=2, space="PSUM") as bpsum,
            ):
                # ---- Phase A: ht -> [fp8 | fp8 residual] partition-major tables.
                # Order: lo chunks, then A2 (hownT, needed by the first
                # epilogues), then hi chunks.  nfT loads ride the sync queue;
                # table writes ride the scalar queue so they are not stuck
                # behind later loads.
                htab_writes = [[], []]

                def phase_a_chunk(ci):
                    n0 = ci * CH
                    P = min(CH, N - n0)
                    nsub = (P + 127) // 128
                    nf_t = apool.tile([128, 2, CH], fh, tag="nf")
                    nc.sync.dma_start(
                        out=nf_t[:, :, :P],
                        in_=nfT_p[:, n0 : n0 + P].rearrange("(k p) n -> p k n", p=128),
                    )
                    hb = apool.tile([128, CH // 128, 2 * D], fp8, tag="hb")
                    for g0 in range(0, nsub, 4):
                        gw = min(4, nsub - g0)
                        ps = apsum.tile([128, 4, D], f32, tag="psA")
                        for s in range(g0, g0 + gw):
                            sp = min(128, P - s * 128)
                            for k in range(2):
                                nc.tensor.matmul(
                                    ps[:sp, s - g0, :],
                                    lhsT=nf_t[:, k, s * 128 : s * 128 + sp],
                                    rhs=Wn_sb[:, k, :],
                                    start=(k == 0),
                                    stop=(k == 1),
                                )
                        nc.scalar.activation(
                            hb[:, g0 : g0 + gw, 0:D], ps[:, :gw, :],
                            mybir.ActivationFunctionType.Copy,
                        )
                        # residual = h_f32 - fp8(h)  (restores ~fp16 accuracy;
                        # GPSIMD cannot read PSUM, so this stays on DVE)
                        nc.vector.tensor_tensor(
                            out=hb[:, g0 : g0 + gw, D : 2 * D],
                            in0=ps[:, :gw, :],
                            in1=hb[:, g0 : g0 + gw, 0:D],
                            op=mybir.AluOpType.subtract,
                        )
                    if ci < NLO // CH:
                        dst_ap = htab[0][:].rearrange("(p t) d -> p t d", p=128)[
                            :, ci * (CH // 128) : ci * (CH // 128) + nsub, :
                        ]
                    else:
                        t0 = (ci - NLO // CH) * (CH // 128)
                        dst_ap = htab[1][:].rearrange("(p t) d -> p t d", p=128)[
                            :, t0 : t0 + nsub, :
                        ]
                    w = nc.scalar.dma_start(out=dst_ap, in_=hb[:, :nsub, :])
                    htab_writes[0 if ci < NLO // CH else 1].append(w)

                for ci in range(NLO // CH):
                    phase_a_chunk(ci)

                # ---- Phase A2: ht.T of own nodes (fp16, transposed layout) ----
                for ci in range(NPC // CH2):
                    n0 = ci * CH2
                    nfo = apool.tile([128, 2, CH2], fh, tag="nfo")
                    nc.sync.dma_start(
                        out=nfo[:],
                        in_=nfTo_p[:, n0 : n0 + CH2].rearrange("(k p) n -> p k n", p=128),
                    )
                    ps2 = apsum.tile([128, 4, BLK], f32, tag="psA2", bufs=1)
                    for k in range(2):
                        nc.tensor.matmul(
                            ps2[:],
                            lhsT=Wn_sb[:, k, :],
                            rhs=nfo[:, k, :],
                            start=(k == 0),
                            stop=(k == 1),
                        )
                    nc.scalar.activation(
                        hownT[:, ci * 4 : ci * 4 + 4, :], ps2[:],
                        mybir.ActivationFunctionType.Copy,
                    )

                # eft loader (scalar queue); prefetch the first PF superblocks
                # ahead of the hi-table writes so sb0's matmuls aren't starved
                eft_pending = {}

                def load_eft(sb):
                    Tspan = sb_T[0][sb] + sb_T[1][sb]
                    goff = span_off[(sb, 0)]
                    t = bpool.tile([128, TSBMAX, D], fp8, tag="eft")
                    nc.scalar.dma_start(
                        out=t[:, :Tspan, :], in_=ef_p[:, goff : goff + Tspan, :]
                    )
                    eft_pending[sb] = t

                PF = 3
                for sb in range(PF):
                    load_eft(sb)

                for ci in range(NLO // CH, NCH):
                    phase_a_chunk(ci)

                # ---- Phase B: one pass, per-superblock lo+hi gathers ----
                qi = 0
                for sb in range(NSB):
                    blocks = [sb * SBLK + j for j in range(SBLK)]
                    Tlo, Thi = sb_T[0][sb], sb_T[1][sb]
                    Tspan = Tlo + Thi
                    goff = span_off[(sb, 0)]
                    assert span_off[(sb, 1)] == goff + Tlo

                    if sb + PF < NSB:
                        load_eft(sb + PF)
                    if sb not in eft_pending:
                        load_eft(sb)
                    eft = eft_pending.pop(sb)
                    gl = bpool.tile([128, TSBMAX, 2 * D], fp8, tag="gl")
                    for h, (c0, cn) in enumerate([(0, Tlo), (Tlo, Thi)]):
                        done = 0
                        while done < cn:
                            ch = min(cn - done, GT)
                            g = nc.gpsimd.dma_gather(
                                gl[:, c0 + done : c0 + done + ch, :],
                                htab[h][:],
                                gidx_sb[:, (goff + c0 + done) * 8 : (goff + c0 + done + ch) * 8],
                                ch * 128, ch * 128, 2 * D,
                                queue_num=qi % NQ,
                            )
                            qi += 1
                            for wi in htab_writes[h]:
                                add_dep_helper(g.ins, wi.ins, reason="gather after table writes")
                            done += ch

                    accs = {}
                    for j, b in enumerate(blocks):
                        accs[b] = bpsum.tile(
                            [128, 256], f32, tag=f"acc{j}", name=f"acc{j}_{sb}"
                        )
                    # stream position -> (block, run id); runs are
                    # (b0,lo),(b1,lo),(b0,hi),(b1,hi)
                    tile_run = []
                    first_of_block = {}
                    for h in range(2):
                        for j, b in enumerate(blocks):
                            tcount = T_list[b][h]
                            for u in range(tcount):
                                if b not in first_of_block:
                                    first_of_block[b] = len(tile_run)
                                tile_run.append((b, h * SBLK + j))

                    for g0 in range(0, Tspan, SGRP):
                        w = min(SGRP, Tspan - g0)
                        S8 = spool.tile([128, SGRP, 128], fp8, tag="S", name="S8")
                        nc.vector.tensor_tensor(
                            out=S8[:, :w, :],
                            in0=dstf_sb[
                                :, goff + g0 : goff + g0 + w, None
                            ].to_broadcast([128, w, 128]),
                            in1=iota8[:, :w, :],
                            op=mybir.AluOpType.is_equal,
                        )
                        t = g0
                        while t < g0 + w:
                            b, run = tile_run[t]
                            pair = (
                                t + 1 < g0 + w
                                and tile_run[t + 1][1] == run
                            )
                            first = first_of_block.get(b) == t
                            jj = t - g0
                            if pair:
                                nc.tensor.matmul(
                                    accs[b][:, 0:128],
                                    lhsT=gl[:, t : t + 2, 0:D],
                                    rhs=S8[:, jj : jj + 2, :],
                                    start=first, stop=False, skip_group_check=True,
                                    perf_mode=DR,
                                )
                                nc.tensor.matmul(
                                    accs[b][:, 0:128],
                                    lhsT=gl[:, t : t + 2, D : 2 * D],
                                    rhs=S8[:, jj : jj + 2, :],
                                    start=False, stop=False, skip_group_check=True,
                                    perf_mode=DR,
                                )
                                nc.tensor.matmul(
                                    accs[b][:, 128:256],
                                    lhsT=eft[:, t : t + 2, :],
                                    rhs=S8[:, jj : jj + 2, :],
                                    start=False, stop=False, skip_group_check=True,
                                    perf_mode=DR,
                                )
                                t += 2
                            else:
                                nc.tensor.matmul(
                                    accs[b][:, 0:128],
                                    lhsT=gl[:, t, 0:D],
                                    rhs=S8[:, jj, :],
                                    start=first, stop=False, skip_group_check=True,
                                )
                                nc.tensor.matmul(
                                    accs[b][:, 0:128],
                                    lhsT=gl[:, t, D : 2 * D],
                                    rhs=S8[:, jj, :],
                                    start=False, stop=False, skip_group_check=True,
                                )
                                nc.tensor.matmul(
                                    accs[b][:, 128:256],
                                    lhsT=eft[:, t, :],
                                    rhs=S8[:, jj, :],
                                    start=False, stop=False, skip_group_check=True,
                                )
                                t += 1

                    # ---- per-block epilogue ----
                    osb = opool.tile([128, SBLK, BLK], fh, tag="osb")
                    for j, b in enumerate(blocks):
                        acc = accs[b]
                        SefT = cpl.tile([128, BLK], fh, tag="SefT")
                        nc.scalar.activation(
                            SefT[:], acc[:, 128 : 128 + BLK],
                            mybir.ActivationFunctionType.Copy,
                        )
                        # acc_hT[:, :125] += We.T @ SefT  (same open group)
                        nc.tensor.matmul(
                            acc[:, 0:BLK], lhsT=We_sb[:], rhs=SefT[:],
                            start=False, stop=True, skip_group_check=True,
                        )
                        aggT = cpl.tile([128, BLK], fh, tag="aggT")
                        nc.vector.tensor_tensor(
                            out=aggT[:], in0=acc[:, 0:BLK],
                            in1=rcolR_sb[:, b, :BLK],
                            op=mybir.AluOpType.mult,
                        )
                        poT = bpsum.tile([128, BLK], f32, tag="poT", bufs=1)
                        nc.tensor.matmul(
                            poT[:], lhsT=Wc1_sb[:], rhs=hownT[:, b, :],
                            start=True, stop=False,
                        )
                        nc.tensor.matmul(
                            poT[:], lhsT=Wc2_sb[:], rhs=aggT[:],
                            start=False, stop=False,
                        )
                        nc.tensor.matmul(
                            poT[:], lhsT=L2_sb[:], rhs=R2_sb[:, b, :BLK],
                            start=False, stop=True,
                        )
                        nc.vector.tensor_copy(osb[:, j, :], poT[:])
                    nc.sync.dma_start(
                        out=outT_p[:, sb * SBLK * BLK : (sb + 1) * SBLK * BLK],
                        in_=osb[:],
                    )

    nc.finalize()
    return nc


def kernel(node_feat, edge_feat, Wn, bn, We, be, Wc, bc, src, dst):
    global LAST_EXEC_NS, LAST_RESULTS
    node_feat = np.asarray(node_feat, np.float32)
    edge_feat = np.asarray(edge_feat, np.float32)
    Wn = np.asarray(Wn, np.float32)
    bn = np.asarray(bn, np.float32)
    We = np.asarray(We, np.float32)
    be = np.asarray(be, np.float32)
    Wc = np.asarray(Wc, np.float32)
    bc = np.asarray(bc, np.float32)
    src = np.asarray(src).astype(np.int64)
    dst = np.asarray(dst).astype(np.int64)

    # ---- host-side edge sharding / ordering ----
    cid = dst // NPC
    rel = dst - cid * NPC
    blk = rel // BLK
    dl = (rel - blk * BLK).astype(np.int64)
    sbi = blk // SBLK
    jin = blk - sbi * SBLK
    half = (src >= NLO).astype(np.int64)
    # stream order: (core, superblock, half, block-within-superblock)
    group = ((cid * NSB + sbi) * 2 + half) * SBLK + jin
    order = np.argsort(group, kind="stable")
    counts = np.bincount(group, minlength=C * NSB * 2 * SBLK).reshape(C, NSB, 2, SBLK)
    tcnt = (counts + 127) // 128
    Tmax = tcnt.max(axis=0)                      # [NSB, 2, SBLK]
    assert (Tmax > 0).all(), "empty (superblock, half, block) group"
    T_list = [
        (int(Tmax[b // SBLK, 0, b % SBLK]), int(Tmax[b // SBLK, 1, b % SBLK]))
        for b in range(NB)
    ]
    T_tot = int(Tmax.sum())
    L = T_tot * 128

    deg = np.bincount(dst, minlength=N).astype(np.float32)
    rcol_all = 1.0 / np.maximum(deg, 1.0)
    mcol_all = np.minimum(deg, 1.0)

    # remapped gather row index (partition-major table layout)
    gmap = np.where(
        src < NLO,
        (src % 128) * TLO + src // 128,
        ((src - NLO) % 128) * THI + (src - NLO) // 128,
    ).astype(np.int16)

    ef_h = edge_feat.astype(f8)

    # slot offsets in stream order (sb-major, then half, then block)
    gstart = np.zeros(C * NSB * 2 * SBLK + 1, np.int64)
    np.cumsum(counts.ravel(), out=gstart[1:])
    slot_off = np.zeros(NSB * 2 * SBLK + 1, np.int64)
    np.cumsum(Tmax.ravel() * 128, out=slot_off[1:])

    bnbeWc2 = (bn + be) @ Wc[D:]
    bias0 = bn @ Wc[:D] + bc
    L2 = np.stack([bnbeWc2, bias0]).astype(f16)

    in_maps = []
    shared = {
        "nfT": np.ascontiguousarray(node_feat.T.astype(f16)),
        "Wn16": Wn.astype(f16),
        "We16": We.astype(f16),
        "Wc116": np.ascontiguousarray(Wc[:D]).astype(f16),
        "Wc216": np.ascontiguousarray(Wc[D:]).astype(f16),
        "L2": L2,
    }
    for c in range(C):
        gidx = np.zeros(L, np.int16)
        dstl = np.full(L, PAD_COL, np.float16)
        eids = np.full(L, -1, np.int64)
        for g_local in range(NSB * 2 * SBLK):
            g = c * (NSB * 2 * SBLK) + g_local
            n = counts.ravel()[g]
            s0 = gstart[g]
            o0 = slot_off[g_local]
            ed = order[s0 : s0 + n]
            gidx[o0 : o0 + n] = gmap[ed]
            dstl[o0 : o0 + n] = dl[ed].astype(np.float16)
            eids[o0 : o0 + n] = ed
        ef_rows = np.zeros((L, D), f8)
        real = eids >= 0
        ef_rows[real] = ef_h[eids[real]]
        R2 = np.zeros((2, NB, 128), f16)
        rcolR = np.zeros((NB, 128), np.float16)
        for b in range(NB):
            n0 = c * NPC + b * BLK
            R2[0, b, :BLK] = mcol_all[n0 : n0 + BLK]
            R2[1, b, :BLK] = 1.0
            rcolR[b, :BLK] = rcol_all[n0 : n0 + BLK]
        rcolR_full = np.ascontiguousarray(
            np.broadcast_to(rcolR.reshape(1, NB * 128), (128, NB * 128))
        )
        in_maps.append(
            dict(
                shared,
                nfTo=np.ascontiguousarray(
                    node_feat.T[:, c * NPC : (c + 1) * NPC].astype(f16)
                ),
                R2=R2.reshape(2, NB * 128),
                rcolR=rcolR_full,
                gidx=_wrap_idx16(gidx),
                dstf=np.ascontiguousarray(dstl.reshape(T_tot, 128).T),
                ef=np.ascontiguousarray(
                    ef_rows.reshape(T_tot, 128, D).transpose(1, 0, 2)
                ),
            )
        )

    nc = _build_graph(T_list)
    res = run_bass_kernel_spmd(nc, in_maps, core_ids=list(range(C)))
    LAST_EXEC_NS = res.exec_time_ns
    LAST_RESULTS = res
    out = np.concatenate(
        [
            np.ascontiguousarray(res.results[c]["outT"].T).astype(np.float32)
            for c in range(C)
        ],
        axis=0,
    )
    return out


# revision 18
# speedup vs baseline: 1.1673x; 1.1673x over previous
"""BasicGNNConv on 8 TRN2 NeuronCores (Bass/Tile).

Math (reference):
    h   = node_feat @ Wn + bn                    # [N, 128]
    e   = edge_feat @ We + be                    # [E, 128]
    m   = h[src] + e
    agg = segment_sum(m, dst) / max(deg, 1)
    out = concat([h, agg]) @ Wc + bc

Linearity rewrite (eliminates all per-edge matmuls; biases folded):
    ht   = node_feat @ Wn                        # no bias
    aggT = rcol * [ (sum ht[src])T + We.T @ (sum ef)T ]      # [feat, slot]
    outT = Wc1.T @ htT_own + Wc2.T @ aggT + bnbeWc2 (x) mcol + bias0 (x) 1

Sharding: edges are assigned to the core that owns their dst node range
(5000 nodes/core) -> per-core segment sums are complete, no collective needed.

v2 changes vs the 395us baseline (trace-driven):
  * Gather batching: the GpSimd DMAGatherAnt descriptor generation was the
    phase-B bottleneck (267us busy, ~90% of the phase-B window) because each
    call carried <=1024 descriptors against a ~1us fixed overhead.  The
    SWDGE carveout is raised to 49152B (3072-desc ring per queue) and each
    (superblock, half) issues ONE gather call (~2.2k descs), lo on queue 0
    and hi on queue 1: 40 calls instead of ~120.
  * fp8 phase B: table rows are [fp8(h) | fp8(h - fp8(h))] (value+residual,
    256B -- the same gather size as fp16, since 256B is the dma_gather
    minimum), and the edge-feature stream is fp8 (halves that stream).  The
    per-tile segment-sum matmuls run as fp8 DoubleRow (2 edge-tiles per
    pass): 3 matmuls (h-val, h-res, ef) per PAIR of tiles.  The residual
    term restores the h path to ~fp16 accuracy; measured end-to-end rel err
    ~1e-2 vs the 2e-2 gate.
  * Merged lo/hi passes: one sweep over 20 superblocks; each superblock
    gathers its lo-src edges (from the lo table) and hi-src edges (hi
    table) back to back, so the lo-partial flush/reload of the two-pass
    scheme disappears.
  * outT is fp16 (host upcasts); rcolR is fp16.

(Collectives were probed for a sharded phase A -- an intra-chip AllGather
completes ~90us after trigger in this runtime, erasing the benefit, so the
table build stays replicated per-core.)
"""
import numpy as np
import ml_dtypes

import concourse.bacc as bacc
import concourse.mybir as mybir
import concourse.tile as tile
from concourse.tile_rust import add_dep_helper
from concourse.bass_utils import run_bass_kernel_spmd

N = 40000
E = 640000
D = 128          # OUT_DIM == EDGE_DIM
ND = 256         # NODE_DIM
C = 8            # cores
NPC = N // C     # 5000 nodes per core
BLK = 125        # nodes per dst block
NB = NPC // BLK  # 40 blocks per core
SBLK = 2         # blocks per superblock
NSB = NB // SBLK
NLO = 16384      # nodes in the lo gather table (8 phase-A chunks)
TLO = NLO // 128          # 128 t-columns
NHICAP = 23680            # 185 * 128 (capacity; real nodes 23616)
THI = NHICAP // 128       # 185
CH = 2048        # phase A chunk (nodes)
NCH = (N + CH - 1) // CH  # 20 (last chunk 1088 nodes)
CH2 = 500        # phase A2 chunk (own nodes)
PAD_COL = 127    # trash column in the 128-wide S window (>= BLK)
NQ = 4           # SWDGE queues
SCRATCH = 16384  # dynamic DMA scratch (HW ring is 1024 descs regardless)
SGRP = 8         # S tiles per IS_EQ call
GT = 8           # tiles per gather call (1024 descs = ring size; GpSimd time
                 # is ~2.7ns/desc generation + ~0.4us/call, so fewer+bigger
                 # calls win: GT sweep measured 8:373us, 4:398us, 2:424us)

LAST_EXEC_NS = None
LAST_RESULTS = None

f16 = np.float16
f8 = ml_dtypes.float8_e4m3    # TRN fp8e4 flavor


def _wrap_idx16(arr):
    """[L] -> [128, L//16] int16 wrapped layout (pos i at [i%16, i//16]),
    replicated across the 8 GPSIMD core partition groups."""
    w = arr.astype(np.int16).reshape(-1, 16).T
    return np.ascontiguousarray(np.tile(w, (8, 1)))


def _build_graph(T_list):
    """T_list[b] = (T_lo, T_hi) tiles per block; stream order is
    (superblock, half, block-within-superblock)."""
    nc = bacc.Bacc(
        None, target_bir_lowering=False, debug=False,
        num_swdge_queues=NQ, dynamic_dma_scratch_size=SCRATCH,
    )
    f32, i16, fh = mybir.dt.float32, mybir.dt.int16, mybir.dt.float16
    fp8 = mybir.dt.float8e4
    DR = mybir.MatmulPerfMode.DoubleRow

    T_tot = sum(tl + th for tl, th in T_list)
    L = T_tot * 128

    nfT_p = nc.declare_dram_parameter("nfT", [ND, N], fh, isOutput=False)
    nfTo_p = nc.declare_dram_parameter("nfTo", [ND, NPC], fh, isOutput=False)
    Wn_p = nc.declare_dram_parameter("Wn16", [ND, D], fh, isOutput=False)
    We_p = nc.declare_dram_parameter("We16", [D, D], fh, isOutput=False)
    Wc1_p = nc.declare_dram_parameter("Wc116", [D, D], fh, isOutput=False)
    Wc2_p = nc.declare_dram_parameter("Wc216", [D, D], fh, isOutput=False)
    L2_p = nc.declare_dram_parameter("L2", [2, D], fh, isOutput=False)
    R2_p = nc.declare_dram_parameter("R2", [2, NB * 128], fh, isOutput=False)
    gidx_p = nc.declare_dram_parameter("gidx", [128, L // 16], i16, isOutput=False)
    dstf_p = nc.declare_dram_parameter("dstf", [128, T_tot], fh, isOutput=False)
    rcolR_p = nc.declare_dram_parameter("rcolR", [128, NB * 128], fh, isOutput=False)
    ef_p = nc.declare_dram_parameter("ef", [128, T_tot, D], fp8, isOutput=False)
    outT_p = nc.declare_dram_parameter("outT", [D, NPC], fh, isOutput=True)

    # table rows are 256B: [fp8 h | fp8 residual]
    htab = [
        nc.dram_tensor("htab_lo", [NLO, 2 * D], fp8),
        nc.dram_tensor("htab_hi", [NHICAP, 2 * D], fp8),
    ]

    # span bookkeeping: stream is (sb, half, block-in-sb)
    sb_T = [[0] * NSB, [0] * NSB]
    for h in range(2):
        for sb in range(NSB):
            sb_T[h][sb] = sum(T_list[sb * SBLK + j][h] for j in range(SBLK))
    TSBMAX = max(sb_T[0][sb] + sb_T[1][sb] for sb in range(NSB))
    # offset of (sb, half) span in the global tile stream
    span_off = {}
    off = 0
    for sb in range(NSB):
        for h in range(2):
            span_off[(sb, h)] = off
            off += sb_T[h][sb]
    assert off == T_tot

    with tile.TileContext(nc) as tc:
        with (
            tc.tile_pool(name="const", bufs=1) as cpool,
            tc.tile_pool(name="tabs", bufs=1) as tpool,
        ):
            # ---- constants / weights in SBUF ----
            iota_i = cpool.tile([128, 128], mybir.dt.int32)
            nc.gpsimd.iota(iota_i[:], pattern=[[1, 128]], base=0, channel_multiplier=0)
            iota8 = cpool.tile([128, SGRP, 128], fh)
            for jj in range(SGRP):
                nc.vector.tensor_copy(iota8[:, jj, :], iota_i[:])

            Wn_sb = cpool.tile([128, ND // 128, D], fh)
            nc.sync.dma_start(out=Wn_sb[:], in_=Wn_p[:].rearrange("(k p) d -> p k d", p=128))
            gidx_sb = cpool.tile([128, L // 16], i16)
            nc.sync.dma_start(out=gidx_sb[:], in_=gidx_p[:])
            dstf_sb = cpool.tile([128, T_tot], fh)
            nc.sync.dma_start(out=dstf_sb[:], in_=dstf_p[:])
            We_sb = cpool.tile([128, D], fh)
            nc.scalar.dma_start(out=We_sb[:], in_=We_p[:])
            Wc1_sb = cpool.tile([128, D], fh)
            nc.scalar.dma_start(out=Wc1_sb[:], in_=Wc1_p[:])
            Wc2_sb = cpool.tile([128, D], fh)
            nc.scalar.dma_start(out=Wc2_sb[:], in_=Wc2_p[:])
            L2_sb = cpool.tile([2, D], fh)
            nc.scalar.dma_start(out=L2_sb[:], in_=L2_p[:])
            R2_sb = cpool.tile([2, NB, 128], fh)
            nc.scalar.dma_start(out=R2_sb[:], in_=R2_p[:].rearrange("p (b j) -> p b j", j=128))
            rcolR_sb = cpool.tile([128, NB, 128], fh)
            nc.scalar.dma_start(
                out=rcolR_sb[:], in_=rcolR_p[:].rearrange("p (b j) -> p b j", j=128)
            )

            hownT = tpool.tile([128, NB, BLK], fh)   # ht.T of own nodes

            with (
                tc.tile_pool(name="phA", bufs=3) as apool,
                tc.tile_pool(name="psA", bufs=2, space="PSUM") as apsum,
                tc.tile_pool(name="phB", bufs=3) as bpool,
                tc.tile_pool(name="phS", bufs=3) as spool,
                tc.tile_pool(name="phC", bufs=2) as cpl,
                tc.tile_pool(name="phO", bufs=2) as opool,
                tc.tile_pool(name="psB", bufs=2, space="PSUM") as bpsum,
            ):
                # ---- Phase A: ht -> [fp8 | fp8 residual] partition-major tables.
                # Order: lo chunks, then A2 (hownT, needed by the first
                # epilogues), then hi chunks.  nfT loads ride the sync queue;
                # table writes ride the scalar queue so they are not stuck
                # behind later loads.
                htab_writes = [[], []]

                def phase_a_chunk(ci):
                    n0 = ci * CH
                    P = min(CH, N - n0)
                    nsub = (P + 127) // 128
                    nf_t = apool.tile([128, 2, CH], fh, tag="nf")
                    nc.sync.dma_start(
                        out=nf_t[:, :, :P],
                        in_=nfT_p[:, n0 : n0 + P].rearrange("(k p) n -> p k n", p=128),
                    )
                    hb = apool.tile([128, CH // 128, 2 * D], fp8, tag="hb")
                    for g0 in range(0, nsub, 4):
                        gw = min(4, nsub - g0)
                        ps = apsum.tile([128, 4, D], f32, tag="psA")
                        for s in range(g0, g0 + gw):
                            sp = min(128, P - s * 128)
                            for k in range(2):
                                nc.tensor.matmul(
                                    ps[:sp, s - g0, :],
                                    lhsT=nf_t[:, k, s * 128 : s * 128 + sp],
                                    rhs=Wn_sb[:, k, :],
                                    start=(k == 0),
                                    stop=(k == 1),
                                )
                        nc.scalar.activation(
                            hb[:, g0 : g0 + gw, 0:D], ps[:, :gw, :],
                            mybir.ActivationFunctionType.Copy,
                        )
                        # residual = h_f32 - fp8(h)  (restores ~fp16 accuracy;
                        # GPSIMD cannot read PSUM, so this stays on DVE)
                        nc.vector.tensor_tensor(
                            out=hb[:, g0 : g0 + gw, D : 2 * D],
                            in0=ps[:, :gw, :],
                            in1=hb[:, g0 : g0 + gw, 0:D],
                            op=mybir.AluOpType.subtract,
                        )
                    if ci < NLO // CH:
                        dst_ap = htab[0][:].rearrange("(p t) d -> p t d", p=128)[
                            :, ci * (CH // 128) : ci * (CH // 128) + nsub, :
                        ]
                    else:
                        t0 = (ci - NLO // CH) * (CH // 128)
                        dst_ap = htab[1][:].rearrange("(p t) d -> p t d", p=128)[
                            :, t0 : t0 + nsub, :
                        ]
                    w = nc.scalar.dma_start(out=dst_ap, in_=hb[:, :nsub, :])
                    htab_writes[0 if ci < NLO // CH else 1].append(w)

                for ci in range(NLO // CH):
                    phase_a_chunk(ci)

                # ---- Phase A2: ht.T of own nodes (fp16, transposed layout) ----
                for ci in range(NPC // CH2):
                    n0 = ci * CH2
                    nfo = apool.tile([128, 2, CH2], fh, tag="nfo")
                    nc.sync.dma_start(
                        out=nfo[:],
                        in_=nfTo_p[:, n0 : n0 + CH2].rearrange("(k p) n -> p k n", p=128),
                    )
                    ps2 = apsum.tile([128, 4, BLK], f32, tag="psA2", bufs=1)
                    for k in range(2):
                        nc.tensor.matmul(
                            ps2[:],
                            lhsT=Wn_sb[:, k, :],
                            rhs=nfo[:, k, :],
                            start=(k == 0),
                            stop=(k == 1),
                        )
                    nc.scalar.activation(
                        hownT[:, ci * 4 : ci * 4 + 4, :], ps2[:],
                        mybir.ActivationFunctionType.Copy,
                    )

                # eft loader (scalar queue); prefetch the first PF superblocks
                # ahead of the hi-table writes so sb0's matmuls aren't starved
                eft_pending = {}

                def load_eft(sb):
                    Tspan = sb_T[0][sb] + sb_T[1][sb]
                    goff = span_off[(sb, 0)]
                    t = bpool.tile([128, TSBMAX, D], fp8, tag="eft")
                    nc.scalar.dma_start(
                        out=t[:, :Tspan, :], in_=ef_p[:, goff : goff + Tspan, :]
                    )
                    eft_pending[sb] = t

                PF = 3
                for sb in range(PF):
                    load_eft(sb)

                for ci in range(NLO // CH, NCH):
                    phase_a_chunk(ci)

                # ---- Phase B: one pass, per-superblock lo+hi gathers ----
                qi = 0
                for sb in range(NSB):
                    blocks = [sb * SBLK + j for j in range(SBLK)]
                    Tlo, Thi = sb_T[0][sb], sb_T[1][sb]
                    Tspan = Tlo + Thi
                    goff = span_off[(sb, 0)]
                    assert span_off[(sb, 1)] == goff + Tlo

                    if sb + PF < NSB:
                        load_eft(sb + PF)
                    if sb not in eft_pending:
                        load_eft(sb)
                    eft = eft_pending.pop(sb)
                    gl = bpool.tile([128, TSBMAX, 2 * D], fp8, tag="gl")
                    for h, (c0, cn) in enumerate([(0, Tlo), (Tlo, Thi)]):
                        done = 0
                        while done < cn:
                            ch = min(cn - done, GT)
                            g = nc.gpsimd.dma_gather(
                                gl[:, c0 + done : c0 + done + ch, :],
                                htab[h][:],
                                gidx_sb[:, (goff + c0 + done) * 8 : (goff + c0 + done + ch) * 8],
                                ch * 128, ch * 128, 2 * D,
                                queue_num=qi % NQ,
                            )
                            qi += 1
                            for wi in htab_writes[h]:
                                add_dep_helper(g.ins, wi.ins, reason="gather after table writes")
                            done += ch

                    accs = {}
                    for j, b in enumerate(blocks):
                        accs[b] = bpsum.tile(
                            [128, 256], f32, tag=f"acc{j}", name=f"acc{j}_{sb}"
                        )
                    # stream position -> (block, run id); runs are
                    # (b0,lo),(b1,lo),(b0,hi),(b1,hi)
                    tile_run = []
                    first_of_block = {}
                    for h in range(2):
                        for j, b in enumerate(blocks):
                            tcount = T_list[b][h]
                            for u in range(tcount):
                                if b not in first_of_block:
                                    first_of_block[b] = len(tile_run)
                                tile_run.append((b, h * SBLK + j))

                    for g0 in range(0, Tspan, SGRP):
                        w = min(SGRP, Tspan - g0)
                        S8 = spool.tile([128, SGRP, 128], fp8, tag="S", name="S8")
                        nc.vector.tensor_tensor(
                            out=S8[:, :w, :],
                            in0=dstf_sb[
                                :, goff + g0 : goff + g0 + w, None
                            ].to_broadcast([128, w, 128]),
                            in1=iota8[:, :w, :],
                            op=mybir.AluOpType.is_equal,
                        )
                        t = g0
                        while t < g0 + w:
                            b, run = tile_run[t]
                            pair = (
                                t + 1 < g0 + w
                                and tile_run[t + 1][1] == run
                            )
                            first = first_of_block.get(b) == t
                            jj = t - g0
                            if pair:
                                nc.tensor.matmul(
                                    accs[b][:, 0:128],
                                    lhsT=gl[:, t : t + 2, 0:D],
                                    rhs=S8[:, jj : jj + 2, :],
                                    start=first, stop=False, skip_group_check=True,
                                    perf_mode=DR,
                                )
                                nc.tensor.matmul(
                                    accs[b][:, 0:128],
                                    lhsT=gl[:, t : t + 2, D : 2 * D],
                                    rhs=S8[:, jj : jj + 2, :],
                                    start=False, stop=False, skip_group_check=True,
                                    perf_mode=DR,
                                )
                                nc.tensor.matmul(
                                    accs[b][:, 128:256],
                                    lhsT=eft[:, t : t + 2, :],
                                    rhs=S8[:, jj : jj + 2, :],
                                    start=False, stop=False, skip_group_check=True,
                                    perf_mode=DR,
                                )
                                t += 2
                            else:
                                nc.tensor.matmul(
                                    accs[b][:, 0:128],
                                    lhsT=gl[:, t, 0:D],
                                    rhs=S8[:, jj, :],
                                    start=first, stop=False, skip_group_check=True,
                                )
                                nc.tensor.matmul(
                                    accs[b][:, 0:128],
                                    lhsT=gl[:, t, D : 2 * D],
                                    rhs=S8[:, jj, :],
                                    start=False, stop=False, skip_group_check=True,
                                )
                                nc.tensor.matmul(
                                    accs[b][:, 128:256],
                                    lhsT=eft[:, t, :],
                                    rhs=S8[:, jj, :],
                                    start=False, stop=False, skip_group_check=True,
                                )
                                t += 1

                    # ---- per-block epilogue ----
                    osb = opool.tile([128, SBLK, BLK], fh, tag="osb")
                    for j, b in enumerate(blocks):
                        acc = accs[b]
                        SefT = cpl.tile([128, BLK], fh, tag="SefT")
                        nc.scalar.activation(
                            SefT[:], acc[:, 128 : 128 + BLK],
                            mybir.ActivationFunctionType.Copy,
                        )
                        # acc_hT[:, :125] += We.T @ SefT  (same open group)
                        nc.tensor.matmul(
                            acc[:, 0:BLK], lhsT=We_sb[:], rhs=SefT[:],
                            start=False, stop=True, skip_group_check=True,
                        )
                        aggT = cpl.tile([128, BLK], fh, tag="aggT")
                        nc.vector.tensor_tensor(
                            out=aggT[:], in0=acc[:, 0:BLK],
                            in1=rcolR_sb[:, b, :BLK],
                            op=mybir.AluOpType.mult,
                        )
                        poT = bpsum.tile([128, BLK], f32, tag="poT", bufs=1)
                        nc.tensor.matmul(
                            poT[:], lhsT=Wc1_sb[:], rhs=hownT[:, b, :],
                            start=True, stop=False,
                        )
                        nc.tensor.matmul(
                            poT[:], lhsT=Wc2_sb[:], rhs=aggT[:],
                            start=False, stop=False,
                        )
                        nc.tensor.matmul(
                            poT[:], lhsT=L2_sb[:], rhs=R2_sb[:, b, :BLK],
                            start=False, stop=True,
                        )
                        nc.vector.tensor_copy(osb[:, j, :], poT[:])
                    nc.sync.dma_start(
                        out=outT_p[:, sb * SBLK * BLK : (sb + 1) * SBLK * BLK],
                        in_=osb[:],
                    )

    nc.finalize()
    return nc


def kernel(node_feat, edge_feat, Wn, bn, We, be, Wc, bc, src, dst):
    global LAST_EXEC_NS, LAST_RESULTS
    node_feat = np.asarray(node_feat, np.float32)
    edge_feat = np.asarray(edge_feat, np.float32)
    Wn = np.asarray(Wn, np.float32)
    bn = np.asarray(bn, np.float32)
    We = np.asarray(We, np.float32)
    be = np.asarray(be, np.float32)
    Wc = np.asarray(Wc, np.float32)
    bc = np.asarray(bc, np.float32)
    src = np.asarray(src).astype(np.int64)
    dst = np.asarray(dst).astype(np.int64)

    # ---- host-side edge sharding / ordering ----
    cid = dst // NPC
    rel = dst - cid * NPC
    blk = rel // BLK
    dl = (rel - blk * BLK).astype(np.int64)
    sbi = blk // SBLK
    jin = blk - sbi * SBLK
    half = (src >= NLO).astype(np.int64)
    # stream order: (core, superblock, half, block-within-superblock)
    group = ((cid * NSB + sbi) * 2 + half) * SBLK + jin
    order = np.argsort(group, kind="stable")
    counts = np.bincount(group, minlength=C * NSB * 2 * SBLK).reshape(C, NSB, 2, SBLK)
    tcnt = (counts + 127) // 128
    Tmax = tcnt.max(axis=0)                      # [NSB, 2, SBLK]
    assert (Tmax > 0).all(), "empty (superblock, half, block) group"
    T_list = [
        (int(Tmax[b // SBLK, 0, b % SBLK]), int(Tmax[b // SBLK, 1, b % SBLK]))
        for b in range(NB)
    ]
    T_tot = int(Tmax.sum())
    L = T_tot * 128

    deg = np.bincount(dst, minlength=N).astype(np.float32)
    rcol_all = 1.0 / np.maximum(deg, 1.0)
    mcol_all = np.minimum(deg, 1.0)

    # remapped gather row index (partition-major table layout)
    gmap = np.where(
        src < NLO,
        (src % 128) * TLO + src // 128,
        ((src - NLO) % 128) * THI + (src - NLO) // 128,
    ).astype(np.int16)

    ef_h = edge_feat.astype(f8)

    # slot offsets in stream order (sb-major, then half, then block)
    gstart = np.zeros(C * NSB * 2 * SBLK + 1, np.int64)
    np.cumsum(counts.ravel(), out=gstart[1:])
    slot_off = np.zeros(NSB * 2 * SBLK + 1, np.int64)
    np.cumsum(Tmax.ravel() * 128, out=slot_off[1:])

    bnbeWc2 = (bn + be) @ Wc[D:]
    bias0 = bn @ Wc[:D] + bc
    L2 = np.stack([bnbeWc2, bias0]).astype(f16)

    in_maps = []
    shared = {
        "nfT": np.ascontiguousarray(node_feat.T.astype(f16)),
        "Wn16": Wn.astype(f16),
        "We16": We.astype(f16),
        "Wc116": np.ascontiguousarray(Wc[:D]).astype(f16),
        "Wc216": np.ascontiguousarray(Wc[D:]).astype(f16),
        "L2": L2,
    }
    for c in range(C):
        gidx = np.zeros(L, np.int16)
        dstl = np.full(L, PAD_COL, np.float16)
        eids = np.full(L, -1, np.int64)
        for g_local in range(NSB * 2 * SBLK):
            g = c * (NSB * 2 * SBLK) + g_local
            n = counts.ravel()[g]
            s0 = gstart[g]
            o0 = slot_off[g_local]
            ed = order[s0 : s0 + n]
            gidx[o0 : o0 + n] = gmap[ed]
            dstl[o0 : o0 + n] = dl[ed].astype(np.float16)
            eids[o0 : o0 + n] = ed
        ef_rows = np.zeros((L, D), f8)
        real = eids >= 0
        ef_rows[real] = ef_h[eids[real]]
        R2 = np.zeros((2, NB, 128), f16)
        rcolR = np.zeros((NB, 128), np.float16)
        for b in range(NB):
            n0 = c * NPC + b * BLK
            R2[0, b, :BLK] = mcol_all[n0 : n0 + BLK]
            R2[1, b, :BLK] = 1.0
            rcolR[b, :BLK] = rcol_all[n0 : n0 + BLK]
        rcolR_full = np.ascontiguousarray(
            np.broadcast_to(rcolR.reshape(1, NB * 128), (128, NB * 128))
        )
        in_maps.append(
            dict(
                shared,
                nfTo=np.ascontiguousarray(
                    node_feat.T[:, c * NPC : (c + 1) * NPC].astype(f16)
                ),
                R2=R2.reshape(2, NB * 128),
                rcolR=rcolR_full,
                gidx=_wrap_idx16(gidx),
                dstf=np.ascontiguousarray(dstl.reshape(T_tot, 128).T),
                ef=np.ascontiguousarray(
                    ef_rows.reshape(T_tot, 128, D).transpose(1, 0, 2)
                ),
            )
        )

    nc = _build_graph(T_list)
    res = run_bass_kernel_spmd(nc, in_maps, core_ids=list(range(C)))
    LAST_EXEC_NS = res.exec_time_ns
    LAST_RESULTS = res
    out = np.concatenate(
        [
            np.ascontiguousarray(res.results[c]["outT"].T).astype(np.float32)
            for c in range(C)
        ],
        axis=0,
    )
    return out
